# revision 80
# baseline (speedup 1.0000x reference)
"""Trainium2 Bass kernel for DeBERTa DisentangledSelfAttention.

Problem: B=1, N=2048, H=1024, NH=16 heads, dh=64, max_rel=512 (span=512).
Sharding: 2 heads per core x 8 cores (tensor parallel over heads); each
core computes its heads' output columns; host concatenates + adds v_bias
and divides by the softmax denominator row.

Final design (193.8us cost-model time, from a 249.6us starting point):
- Disentangled bias terms are computed as full [q, s] / [k, s2] arrays
  (phase 2), written to DRAM, and read back as per-k-tile diagonal band
  tiles (transpose-DMA for c2p, plain strided for p2c); the +-512 clamp
  is pad columns in the arrays plus far-field factors (exp'd edge-row
  multiplies on E, pre-scaled vext_lo/hi in pass B).
- p2c arrays are fp8; its band-add into the scores PSUM uses an fp8
  DoubleRow matmul with an [identity; zeros] stationary pair and a
  stride-0 rhs duplication (0.5 cycles/col). c2p stays bf16 (the xbar
  transpose DMA requires 2-byte dtypes).
- pos_proj/pos_q_proj projections run as fp8 DoubleRow matmuls over
  host-packed [128, cp, 2, .] operands (positional terms are ~2% of
  score magnitude, so fp8 noise there is invisible).
- posKrev is produced on-chip by reversed-stride evacuation copies
  (relT_rev input eliminated); weights are host-pre-swizzled so every
  load is one contiguous-descriptor DMA.
- Schedule: phase 1 interleaves with phase 2 of head 0 per 512-column
  chunk; phase 2 of head 1 + its band reads + edge factors interleave
  into pass A/B of head 0 (inter_work); pass B of each q-block is
  software-pipelined into pass A of the next block so PE fills the
  exp-latency bubbles. Single PSUM pool (no pool-swap barriers): tag
  mm2 = 3x2-bank slots shared by q/k/v/phase-2/pass-A tiles, tag mm =
  2x1-bank for the rest; evacuation copies are split DVE/Act by
  swept per-array patterns; far-field multiplies stay on DVE; band
  reads issue as early as their DRAM RAW deps allow (first 6 c2p tiles
  from the evac pool, before the band pools can exist).
"""
import math
import numpy as np
import ml_dtypes

import concourse.bass as bass
import concourse.bacc as bacc
import concourse.tile as tile
from concourse import mybir
from concourse.bass_utils import run_bass_kernel_spmd
from concourse.tile_rust import add_dep_helper

bf16 = ml_dtypes.bfloat16
FP32 = mybir.dt.float32
BF16 = mybir.dt.bfloat16
FP8 = mybir.dt.float8e4

N, H, NH, dh = 2048, 1024, 16, 64
S2 = 1024            # 2 * span
PAD = 128
W = S2 + 2 * PAD     # 1280
NW = N * W
SCALE = math.sqrt(dh * 3.0)
NCORES = 8
KT = N // 128        # 16 k-tiles

_compiled = None

import os
EV_PATTERN = os.environ.get("K_EV", "vvav")        # j-loop evac engines
H1_ACT_N = int(os.environ.get("K_H1A", "0"))     # of 4 h1 copies -> Act
FAR_MODE = int(os.environ.get("K_FAR", "0"))     # 0 dve,1 alt,2 pool
C2PD_BUFS = int(os.environ.get("K_C2PD", "10"))
EPOOL_BUFS = int(os.environ.get("K_EPOOL", "18"))
OT_ACT = int(os.environ.get("K_OT", "0"))
EVC = os.environ.get("K_EVC", "va")   # c2p evac pattern (h0); "" = shared ev
EVP = os.environ.get("K_EVP", "av")   # p2c evac pattern (h0)


def band_window(kt):
    k0 = kt * 128
    return max(0, k0 - 512), min(N, k0 + 640)


def build_program():
    nc = bacc.Bacc("TRN2", target_bir_lowering=False, debug=False,
                   num_devices=NCORES)

    # ---------------- I/O ----------------
    hiddenT = nc.dram_tensor("hiddenT", [H, N], BF16, kind="ExternalInput")
    rel8 = nc.dram_tensor("rel8", [128, 4, 2, S2], FP8, kind="ExternalInput")
    Wpp8 = nc.dram_tensor("Wpp8", [128, 2, 4, 2, 128], FP8,
                          kind="ExternalInput")
    Wqkv = nc.dram_tensor("Wqkv", [128, 3, 8, 128], BF16,
                          kind="ExternalInput")
    ident = nc.dram_tensor("ident", [128, 128], BF16, kind="ExternalInput")
    qb = nc.dram_tensor("qb", [128], FP32, kind="ExternalInput")
    bq = nc.dram_tensor("bq", [128], FP32, kind="ExternalInput")
    out = nc.dram_tensor("out", [130, N], FP32, kind="ExternalOutput")

    # internal DRAM scratch for the raw bias arrays (flat for diagonal APs)
    c2p_arr = nc.dram_tensor("c2p_arr", [2 * NW], BF16)
    p2c_arr = nc.dram_tensor("p2c_arr", [2 * NW], FP8)

    def diag_ap(t, offset, ap):
        return bass.AP(tensor=t, offset=offset, ap=[list(p) for p in ap])

    # phase-2 write instructions, for explicit DRAM RAW deps into band reads
    NG = 8
    c2p_wr = [[None] * NG for _ in range(2)]
    p2c_wr = [[None] * NG for _ in range(2)]

    with tile.TileContext(nc) as tc:
        with (
            tc.tile_pool(name="const", bufs=1) as const,
            tc.tile_pool(name="proj", bufs=1) as proj,
            tc.tile_pool(name="work", bufs=2) as work,
            tc.tile_pool(name="evac", bufs=3) as evac,
            tc.tile_pool(name="epool", bufs=10) as epool,
            tc.tile_pool(name="ps", bufs=2, space="PSUM") as psA,
        ):
            # ---------------- load inputs ----------------
            bigin_cm = tc.tile_pool(name="bigin", bufs=1)
            bigin = bigin_cm.__enter__()
            r8 = bigin.tile([128, 4, 2, S2], FP8, tag="r8")
            for cc in range(2):
                nc.gpsimd.dma_start(
                    r8[:, 2*cc:2*cc+2, :, :], rel8.ap()[:, 2*cc:2*cc+2, :, :])
            idt8z = const.tile([128, 2, 128], FP8, tag="idt8z")
            wpp = const.tile([128, 2, 4, 2, 128], FP8, tag="wpp")
            nc.scalar.dma_start(wpp[:], Wpp8.ap())
            wpos, wposq = wpp[:, 0], wpp[:, 1]
            idt = const.tile([128, 128], BF16, tag="idt")
            nc.scalar.dma_start(idt[:], ident.ap())
            nc.vector.memset(idt8z[:, 1, :], 0.0)
            nc.vector.tensor_copy(idt8z[:, 0, :], idt[:])
            hT = bigin.tile([128, 8, N], BF16, tag="hT")
            # first column-chunk split by contraction halves so the j=0
            # q/k matmuls can start on c=0..3 while c=4..7 streams in
            for ch in range(2):
                nc.gpsimd.dma_start(
                    hT[:, 4*ch:4*ch+4, 0:512],
                    hiddenT.ap()[4*ch*128:(4*ch+4)*128, 0:512]
                    .rearrange("(c p) n -> p c n", p=128))
            for j in range(1, 4):
                nc.gpsimd.dma_start(
                    hT[:, :, j*512:(j+1)*512],
                    hiddenT.ap()[:, j*512:(j+1)*512]
                    .rearrange("(c p) n -> p c n", p=128))
            qb128 = const.tile([128, 1], FP32, tag="qb128")
            nc.sync.dma_start(qb128[:], qb.ap().rearrange("(p one) -> p one", one=1))
            bq128 = const.tile([128, 1], FP32, tag="bq128")
            nc.sync.dma_start(bq128[:], bq.ap().rearrange("(p one) -> p one", one=1))
            wqkv = const.tile([128, 3, 8, 128], BF16, tag="wqkv")
            nc.sync.dma_start(wqkv[:, 0:2], Wqkv.ap()[:, 0:2])
            nc.sync.dma_start(wqkv[:, 2:3], Wqkv.ap()[:, 2:3])
            wq, wk, wv = wqkv[:, 0], wqkv[:, 1], wqkv[:, 2]

            zeros128 = const.tile([128, PAD], BF16, tag="zeros128")
            nc.vector.memset(zeros128[:], 0.0)
            ones_row = const.tile([1, 128], BF16, tag="ones_row")
            nc.vector.memset(ones_row[:], 1.0)

            # joint-head projection tiles: h0 rows 0:64, h1 rows 64:128
            qTx = proj.tile([128, N], BF16, tag="qTx", name="qTx")
            kTx = proj.tile([128, N], BF16, tag="kTx", name="kTx")
            posKrev = proj.tile([128, W], BF16, tag="posKrev", name="posKrev")
            posQ = proj.tile([128, W], BF16, tag="posQ", name="posQ")
            # fused-head vext: h0 at cols 0:65, h1 at cols 65:130
            vext = proj.tile([128, KT, 130], BF16, tag="vext", name="vext")
            vext_lo = proj.tile([128, KT, 130], BF16, tag="vextlo", name="vextlo")
            vext_hi = proj.tile([128, KT, 130], BF16, tag="vexthi", name="vexthi")
            expElo = [proj.tile([128, N], BF16, tag=f"expElo{h}", name=f"expElo{h}")
                      for h in range(2)]
            expEhi = [proj.tile([128, N], BF16, tag=f"expEhi{h}", name=f"expEhi{h}")
                      for h in range(2)]
            expF_lo = [proj.tile([128, KT], FP32, tag=f"Flo{h}", name=f"Flo{h}")
                       for h in range(2)]
            expF_hi = [proj.tile([128, KT], FP32, tag=f"Fhi{h}", name=f"Fhi{h}")
                       for h in range(2)]

            c2p_band = [{} for _ in range(2)]
            p2c_big = [None, None]
            ev_engines = [nc.vector, nc.scalar]
            ev_state = [0]

            def ev_copy(dst, src):
                pat = EV_PATTERN
                c = pat[ev_state[0] % len(pat)]
                eng = nc.vector if c == "v" else nc.scalar
                ev_state[0] += 1
                if eng is nc.scalar:
                    eng.copy(dst, src)
                else:
                    eng.tensor_copy(dst, src)

            # ---------- phase 1a: pos projections (posKrev reversed) -------
            for wtile, dst, bias, rev in (
                (wpos, posKrev, None, True),
                (wposq, posQ, bq128, False),
            ):
                ecol = work.tile([128, 2], FP32, tag="ecol", name="ecol")
                for j in range(2):          # forward S2 in 512 cols
                    ps = psA.tile([128, 512], FP32, tag="mm", name="ps_pos")
                    for c in range(4):
                        nc.tensor.matmul(
                            ps[:], wtile[:, c, :, :],
                            r8[:, c, :, j*512:(j+1)*512],
                            start=(c == 0), stop=(c == 3),
                            perf_mode=mybir.MatmulPerfMode.DoubleRow)
                    if rev:
                        # forward col u -> stored col PAD + (S2-1-u)
                        nc.vector.tensor_copy(
                            posKrev[:, PAD + (1-j)*512:PAD + (2-j)*512],
                            ps[:, 511::-1])
                        ecsl = (slice(0, 1) if j == 0 else slice(511, 512))
                        nc.vector.tensor_copy(ecol[:, j:j+1], ps[:, ecsl])
                    else:
                        nc.vector.tensor_scalar_add(
                            posQ[:, PAD + j*512:PAD + (j+1)*512], ps[:],
                            bias[:])
                        ecsl = (slice(0, 1) if j == 0 else slice(511, 512))
                        nc.vector.tensor_scalar_add(
                            ecol[:, j:j+1], ps[:, ecsl], bias[:])
                # edge-replicated pads (stored coordinates)
                lo_e = ecol[:, 1:2] if rev else ecol[:, 0:1]
                hi_e = ecol[:, 0:1] if rev else ecol[:, 1:2]
                nc.vector.tensor_scalar_add(dst[:, 0:PAD], zeros128[:], lo_e)
                nc.vector.tensor_scalar_add(dst[:, PAD+S2:W], zeros128[:], hi_e)

            # ---------- phase 2 emit helper ----------
            def phase2_group(h, tg, dve_only=False):
                hp = slice(h * 64, (h + 1) * 64)
                for arrsel in range(2):
                    src = qTx if arrsel == 0 else kTx
                    pos = posKrev if arrsel == 0 else posQ
                    arr = c2p_arr if arrsel == 0 else p2c_arr
                    wr_list = c2p_wr if arrsel == 0 else p2c_wr
                    et = evac.tile([128, 2, W], BF16 if arrsel == 0 else FP8,
                                   tag="et" if arrsel == 0 else "et8",
                                   name="et", bufs=3)
                    for ti in range(2):
                        t = tg * 2 + ti
                        ps = psA.tile([128, 1024], FP32, tag="mm2", bufs=3,
                                      name="ps_arr")
                        ps2 = psA.tile([128, 256], FP32, tag="mm",
                                       name="ps_arr2")
                        for j0 in (0, 512):
                            nc.tensor.matmul(
                                ps[:, j0:j0+512],
                                src[hp, t*128:(t+1)*128],
                                pos[hp, j0:j0+512], start=True, stop=True,
                                skip_group_check=True)
                        nc.tensor.matmul(
                            ps2[:], src[hp, t*128:(t+1)*128],
                            pos[hp, 1024:1280], start=True, stop=True,
                            skip_group_check=True)
                        pat = EVC if arrsel == 0 else EVP
                        if dve_only:
                            idx = (tg % 2) * 2 + ti
                            eng = nc.scalar if idx < H1_ACT_N else nc.vector
                            if eng is nc.scalar:
                                eng.copy(et[:, ti, 0:1024], ps[:])
                                eng.copy(et[:, ti, 1024:1280], ps2[:])
                            else:
                                eng.tensor_copy(et[:, ti, 0:1024], ps[:])
                                eng.tensor_copy(et[:, ti, 1024:1280], ps2[:])
                        elif pat:
                            c = pat[(tg * 2 + ti) % len(pat)]
                            eng = nc.vector if c == "v" else nc.scalar
                            if eng is nc.scalar:
                                eng.copy(et[:, ti, 0:1024], ps[:])
                                eng.copy(et[:, ti, 1024:1280], ps2[:])
                            else:
                                eng.tensor_copy(et[:, ti, 0:1024], ps[:])
                                eng.tensor_copy(et[:, ti, 1024:1280], ps2[:])
                        else:
                            ev_copy(et[:, ti, 0:1024], ps[:])
                            ev_copy(et[:, ti, 1024:1280], ps2[:])
                    if arrsel == 0:
                        wr = nc.sync.dma_start(
                            diag_ap(arr, h * NW + tg * 256 * W,
                                    [[W, 128], [128 * W, 2], [1, W]]),
                            et[:])
                    else:
                        wr = nc.gpsimd.dma_start(
                            diag_ap(arr, h * NW + tg * 256 * W,
                                    [[W, 128], [128 * W, 2], [1, W]]),
                            et[:])
                    wr_list[h][tg] = wr.ins

            # ---------- phase 1b interleaved with phase 2 of head 0 --------
            for j in range(4):              # N in 512 cols
                for wtile, dst, bias in ((wq, qTx, qb128), (wk, kTx, None)):
                    ps = psA.tile([128, 512], FP32, tag="mm2", bufs=3,
                                  name="ps_p1")
                    for c in range(8):
                        nc.tensor.matmul(
                            ps[:], wtile[:, c, :], hT[:, c, j*512:(j+1)*512],
                            start=(c == 0), stop=(c == 7))
                    if bias is not None:
                        nc.vector.tensor_scalar_add(
                            dst[:, j*512:(j+1)*512], ps[:], bias[:])
                    else:
                        nc.vector.tensor_copy(dst[:, j*512:(j+1)*512], ps[:])
                # phase 2 h0 for the two 256-row groups now available
                phase2_group(0, 2 * j)
                phase2_group(0, 2 * j + 1)

            # ---------- edges + phase2 h1 + edges h1 ----------
            erow_keep = {}

            def edges(h):
                hp = slice(h * 64, (h + 1) * 64)
                for dst, col in ((expElo[h], PAD + S2 - 1), (expEhi[h], PAD)):
                    erow = work.tile([1, N], BF16, tag="erow", name="erow",
                                     bufs=4)
                    erow_keep[(h, 0 if dst is expElo[h] else 1)] = erow
                    for j in range(4):
                        ps = psA.tile([1, 512], FP32, tag="mm", name="ps_er")
                        nc.tensor.matmul(
                            ps[:], posKrev[hp, col:col+1],
                            qTx[hp, j*512:(j+1)*512],
                            start=True, stop=True)
                        nc.vector.tensor_copy(erow[:, j*512:(j+1)*512], ps[:])
                    for j in range(4):
                        psb = psA.tile([128, 512], FP32, tag="mm", name="ps_eb")
                        nc.tensor.matmul(
                            psb[:], ones_row[:], erow[:, j*512:(j+1)*512],
                            start=True, stop=True)
                        nc.scalar.activation(
                            dst[:, j*512:(j+1)*512], psb[:],
                            mybir.ActivationFunctionType.Exp)
                for dst, col in ((expF_lo[h], PAD), (expF_hi[h], PAD + S2 - 1)):
                    ps = psA.tile([128, KT], FP32, tag="mm", name="ps_f")
                    for t in range(KT):
                        nc.tensor.matmul(
                            ps[:, t:t+1], kTx[hp, t*128:(t+1)*128],
                            posQ[hp, col:col+1], start=True, stop=True)
                    nc.scalar.activation(
                        dst[:], ps[:], mybir.ActivationFunctionType.Exp)
                hc = slice(h * 65, h * 65 + 65)
                for kt in range(KT):
                    (nc.gpsimd if kt % 2 else nc.vector).tensor_scalar_mul(
                        vext_lo[:, kt, hc], vext[:, kt, hc],
                        expF_lo[h][:, kt:kt+1])
                    (nc.vector if kt % 2 else nc.gpsimd).tensor_scalar_mul(
                        vext_hi[:, kt, hc], vext[:, kt, hc],
                        expF_hi[h][:, kt:kt+1])

            # early c2p band reads (kt0-5, head 0) from the evac pool so
            # they issue before the band pools exist
            for kt in range(8):
                k0 = kt * 128
                wlo, whi = band_window(kt)
                run = whi - wlo
                c2p_t = evac.tile([128, 1152], BF16, tag="c2pe",
                                  name="c2pe", bufs=8)
                B0 = 0 * NW + wlo * (W - 1) + (W // 2 - 1) + k0
                rd = (nc.sync if kt % 2 == 0 else nc.scalar).dma_start(
                    c2p_t[:, 0:run],
                    diag_ap(c2p_arr, B0, [[W - 1, run], [1, 128]]),
                    transpose=True)
                for g in range(wlo // 256, (whi + 255) // 256):
                    add_dep_helper(rd.ins, c2p_wr[0][g],
                                   reason="c2p DRAM RAW")
                c2p_band[0][kt] = c2p_t

            # v-projections (deferred past the j-loop evac congestion)
            for t in range(KT):
                ps = psA.tile([128, 128], FP32, tag="mm2", bufs=3,
                              name="ps_v")
                for c in range(8):
                    nc.tensor.matmul(
                        ps[:], hT[:, c, t*128:(t+1)*128], wv[:, c, :],
                        start=(c == 0), stop=(c == 7))
                ev_copy(vext[:, t, 0:64], ps[:, 0:64])
                ev_copy(vext[:, t, 65:129], ps[:, 64:128])
            nc.vector.memset(vext[:, :, 64:65], 1.0)
            nc.vector.memset(vext[:, :, 129:130], 1.0)
            bigin_cm.__exit__(None, None, None)
            band_cm = tc.tile_pool(name="c2pd", bufs=C2PD_BUFS)
            c2pd = band_cm.__enter__()
            band2_cm = tc.tile_pool(name="p2cd", bufs=2)
            p2cd = band2_cm.__enter__()

            def c2p_read(h, kt):
                k0 = kt * 128
                wlo, whi = band_window(kt)
                run = whi - wlo
                c2p_t = c2pd.tile([128, 1152], BF16, tag="c2p_t",
                                  name="c2p_t")
                B0 = h * NW + wlo * (W - 1) + (W // 2 - 1) + k0
                rd = (nc.sync if kt % 2 == 0 else nc.scalar).dma_start(
                    c2p_t[:, 0:run],
                    diag_ap(c2p_arr, B0, [[W - 1, run], [1, 128]]),
                    transpose=True)
                for g in range(wlo // 256, (whi + 255) // 256):
                    add_dep_helper(rd.ins, c2p_wr[h][g],
                                   reason="c2p DRAM RAW")
                c2p_band[h][kt] = c2p_t

            def p2c_read(h, ktg):
                # two 2-k-tile reads; rows (ktg*4+2i..)*128 = write group 2g+i
                if p2c_big[h] is None:
                    p2c_big[h] = p2cd.tile([128, KT, 1152], FP8, tag="p2c_t",
                                           name="p2c_t")
                for i in range(2):
                    k0t = ktg * 4 + 2 * i
                    C0 = h * NW + k0t * 128 * W + 128
                    rd = nc.gpsimd.dma_start(
                        p2c_big[h][:, k0t:k0t+2, :],
                        diag_ap(p2c_arr, C0,
                                [[W - 1, 128], [128 * W, 2], [1, 1152]]))
                    add_dep_helper(rd.ins, p2c_wr[h][2*ktg + i],
                                   reason="p2c DRAM RAW")

            for g in range(4):
                p2c_read(0, g)
            for kt in range(8, KT):
                c2p_read(0, kt)
            edges(0)

            # h1 phase-2/read/edge work interleaved into passAB(h0):
            # after passA(h0, qs) emit inter_work[qs]
            def h1_tg(tg):
                return lambda: phase2_group(1, tg, dve_only=True)

            inter_work = {
                (0, 0): [h1_tg(0), h1_tg(1),
                         lambda: p2c_read(1, 0)],
                (0, 1): [h1_tg(2), h1_tg(3),
                         lambda: p2c_read(1, 1)]
                        + [(lambda kt: lambda: c2p_read(1, kt))(k)
                           for k in (0, 1)],
                (0, 2): [h1_tg(4), h1_tg(5),
                         lambda: p2c_read(1, 2), lambda: edges(1)]
                        + [(lambda kt: lambda: c2p_read(1, kt))(k)
                           for k in (2, 3, 4, 5)],
                (0, 3): [h1_tg(6), h1_tg(7),
                         lambda: p2c_read(1, 3)]
                        + [(lambda kt: lambda: c2p_read(1, kt))(k)
                           for k in (6, 7, 8, 9, 10, 11, 12, 13, 14, 15)],
            }

            # ============ phase 3: A/B software-pipelined ============
            # pass B of block (h, qs) is emitted interleaved into pass A of
            # the NEXT block, so PE has work while Act drains the exps.
            def passA_group(h, qs, ktp, Es):
                hp = slice(h * 64, (h + 1) * 64)
                q0s, q1s = qs * 512, (qs + 1) * 512
                psp = psA.tile([128, 1024], FP32, tag="mm2", bufs=3,
                               name="ps_s")
                E2 = epool.tile([128, 1024], BF16, tag="E", name="E",
                                bufs=EPOOL_BUFS)
                for ki in range(2):
                    kt = ktp * 2 + ki
                    k0 = kt * 128
                    wlo_f, whi_f = band_window(kt)
                    wlo = min(max(wlo_f, q0s), q1s)
                    whi = min(max(whi_f, q0s), q1s)
                    run = whi - wlo
                    eo = ki * 512
                    nc.tensor.matmul(
                        psp[:, eo:eo+512],
                        kTx[hp, k0:k0+128], qTx[hp, q0s:q1s],
                        start=True, stop=(run <= 0),
                        skip_group_check=True)
                    if run <= 0:
                        continue
                    d0c = wlo - wlo_f            # clamped-window base
                    d0p = wlo - (k0 - 512)       # unclamped base
                    nc.tensor.matmul(
                        psp[:, eo+wlo-q0s:eo+whi-q0s],
                        idt[:], c2p_band[h][kt][:, d0c:d0c+run],
                        start=False, stop=False,
                        skip_group_check=True)
                    p2c_sl = p2c_big[h][:, kt, d0p:d0p+run]
                    p2c_dup = bass.AP(
                        tensor=p2c_sl.tensor, offset=p2c_sl.offset,
                        ap=[list(p2c_sl.ap[0]), [0, 2]] +
                           [list(p) for p in p2c_sl.ap[1:]])
                    nc.tensor.matmul(
                        psp[:, eo+wlo-q0s:eo+whi-q0s],
                        idt8z[:], p2c_dup,
                        start=False, stop=True,
                        perf_mode=mybir.MatmulPerfMode.DoubleRow,
                        skip_group_check=True)
                nc.scalar.activation(
                    E2[:], psp[:], mybir.ActivationFunctionType.Exp)
                for ki in range(2):
                    kt = ktp * 2 + ki
                    wlo_f, whi_f = band_window(kt)
                    wlo = min(max(wlo_f, q0s), q1s)
                    whi = min(max(whi_f, q0s), q1s)
                    eo = ki * 512
                    def _far_eng(i):
                        if FAR_MODE == 0:
                            return nc.vector
                        if FAR_MODE == 2:
                            return nc.gpsimd
                        return nc.vector if (kt + i) % 2 else nc.gpsimd
                    if wlo > q0s:
                        _far_eng(0).tensor_mul(
                            E2[:, eo:eo+wlo-q0s], E2[:, eo:eo+wlo-q0s],
                            expElo[h][:, q0s:wlo])
                    if q1s > whi:
                        _far_eng(1).tensor_mul(
                            E2[:, eo+whi-q0s:eo+512],
                            E2[:, eo+whi-q0s:eo+512],
                            expEhi[h][:, whi:q1s])
                Es.append(E2)

            def passB_chunk(state, kts):
                h, qs, Es, ctx_ps = state
                hc = slice(h * 65, h * 65 + 65)
                q0s, q1s = qs * 512, (qs + 1) * 512
                for kt in kts:
                    wlo_f, whi_f = band_window(kt)
                    wlo = min(max(wlo_f, q0s), q1s)
                    whi = min(max(whi_f, q0s), q1s)
                    eo = (kt % 2) * 512
                    E = Es[kt // 2]
                    segs = []
                    if wlo > q0s:
                        segs.append((q0s, wlo, vext_lo))
                    if whi > wlo:
                        segs.append((wlo, whi, vext))
                    if q1s > whi:
                        segs.append((whi, q1s, vext_hi))
                    for (a, b, vv) in segs:
                        nc.tensor.matmul(
                            ctx_ps[:, a-q0s:b-q0s],
                            vv[:, kt, hc],
                            E[:, eo+a-q0s:eo+b-q0s],
                            start=False,
                            stop=(kt == KT - 1),
                            skip_group_check=True)

            def passB_finish(state):
                h, qs, _, ctx_ps = state
                q0s = qs * 512
                for half in range(2):
                    ot = work.tile([65, 256], FP32, tag="ot", name="ot",
                                   bufs=4)
                    sl = slice(half * 256, (half + 1) * 256)
                    if OT_ACT:
                        nc.scalar.copy(ot[:], ctx_ps[:, sl])
                    else:
                        nc.vector.tensor_copy(ot[:], ctx_ps[:, sl])
                    nc.sync.dma_start(
                        out.ap()[h*65:(h+1)*65,
                                 q0s + half*256:q0s + (half+1)*256], ot[:])

            pending = None
            for h in range(2):
                for qs in range(4):
                    Es = []
                    ctx_ps = psA.tile([65, 512], FP32, tag="mm",
                                      name="ctx_ps")
                    nc.vector.memset(ctx_ps[:], 0.0)
                    for ktp in range(KT // 2):
                        passA_group(h, qs, ktp, Es)
                        if pending is not None:
                            passB_chunk(pending, [2*ktp, 2*ktp + 1])
                    if pending is not None:
                        passB_finish(pending)
                    for fn in inter_work.pop((h, qs), []):
                        fn()
                    pending = (h, qs, Es, ctx_ps)
            for ktp in range(KT // 2):
                passB_chunk(pending, [2*ktp, 2*ktp + 1])
            passB_finish(pending)

            band2_cm.__exit__(None, None, None)
            band_cm.__exit__(None, None, None)

    return nc


def _prep_inputs(hidden_states, rel_embeddings, in_proj_w,
                 q_bias, pos_proj_w, pos_q_proj_w, pos_q_proj_b):
    """Host-side sharding/transposes. Returns per-core input maps."""
    hidden = np.asarray(hidden_states, np.float32)[0]      # [N, H]
    rel = np.asarray(rel_embeddings, np.float32)           # [S2, H]
    in_proj_w = np.asarray(in_proj_w, np.float32)
    q_bias = np.asarray(q_bias, np.float32)
    pos_proj_w = np.asarray(pos_proj_w, np.float32)
    pos_q_proj_w = np.asarray(pos_q_proj_w, np.float32)
    pos_q_proj_b = np.asarray(pos_q_proj_b, np.float32)

    fp8 = ml_dtypes.float8_e4m3fn
    hiddenT = np.ascontiguousarray(hidden.T).astype(bf16)
    relT = rel.T                                           # [H, S2] f32
    rel8 = np.ascontiguousarray(
        relT.reshape(4, 2, 128, S2).transpose(2, 0, 1, 3)).astype(fp8)

    in_maps = []
    for c in range(NCORES):
        heads = (2 * c, 2 * c + 1)
        wqs, wks, wvs, wps, wpqs, qbs, bqs = [], [], [], [], [], [], []
        for h in heads:
            wqs.append(in_proj_w[h*192:h*192+64] / SCALE)
            wks.append(in_proj_w[h*192+64:h*192+128])
            wvs.append(in_proj_w[h*192+128:h*192+192])
            wps.append(pos_proj_w[h*64:(h+1)*64])
            wpqs.append(pos_q_proj_w[h*64:(h+1)*64] / SCALE)
            qbs.append(q_bias[h*64:(h+1)*64] / SCALE)
            bqs.append(pos_q_proj_b[h*64:(h+1)*64] / SCALE)
        def swz(w):
            # [H, 128] (row h = c*128+p) -> [128, 8, 128] (p, c, m)
            wt = np.concatenate(w, 0).T.astype(np.float32)
            return wt.reshape(8, 128, 128).transpose(1, 0, 2)
        def swz8(w):
            # [H, 128] -> [128(p), 4(cp), 2(s), 128(m)], chunk c = cp*2+s
            wt = np.concatenate(w, 0).T.astype(np.float32)
            return wt.reshape(4, 2, 128, 128).transpose(2, 0, 1, 3)
        wpp8 = np.stack([swz8(wps), swz8(wpqs)], axis=1).astype(fp8)
        wqkv = np.stack([swz(wqs), swz(wks), swz(wvs)], axis=1).astype(bf16)
        in_maps.append({
            "ident": np.eye(128, dtype=bf16),
            "hiddenT": hiddenT,
            "rel8": rel8,
            "Wpp8": np.ascontiguousarray(wpp8),
            "Wqkv": np.ascontiguousarray(wqkv),
            "qb": np.concatenate(qbs).astype(np.float32),
            "bq": np.concatenate(bqs).astype(np.float32),
        })
    return in_maps


def kernel(hidden_states, attention_mask, relative_pos, rel_embeddings,
           in_proj_w, q_bias, v_bias, pos_proj_w, pos_q_proj_w, pos_q_proj_b):
    global _compiled
    in_maps = _prep_inputs(hidden_states, rel_embeddings, in_proj_w,
                           q_bias, pos_proj_w, pos_q_proj_w, pos_q_proj_b)
    if _compiled is None:
        _compiled = build_program()
        _compiled.finalize()
    res = run_bass_kernel_spmd(_compiled, in_maps, list(range(NCORES)))
    full = np.empty((N, H), np.float32)
    for c in range(NCORES):
        o = res.results[c]["out"]
        for hh in range(2):
            ctx = o[hh*65:hh*65+64, :]            # [64, N]
            den = o[hh*65+64, :]                  # [N]
            full[:, c*128+hh*64:c*128+(hh+1)*64] = (ctx / den[None, :]).T
    full += np.asarray(v_bias, np.float32)[None, :]
    return full.reshape(1, N, H)


if __name__ == "__main__":
    import ref_np as reference
    inputs = {k: np.asarray(v) for k, v in reference.setup_inputs().items()}
    outp = kernel(**inputs)
    ref = np.asarray(reference.reference(**inputs))
    err = np.abs(outp - ref)
    print(f"max abs err {err.max():.3e}  rel {err.max()/np.abs(ref).max():.3e}")
